# revision 61
# baseline (speedup 1.0000x reference)
"""Banded multi-head attention (window=256) on 8 Trainium2 NeuronCores.

Sharding: core c handles batch b = c // 4 and head group g = c % 4
(4 of 16 heads). QKV projection is column-sharded per head group, the
banded attention is embarrassingly parallel over (batch, head), and the
output projection is row-sharded (each core produces a partial [S, E]
output in fp16; the host sums the 4 partials per batch and adds the
bias).

All matmul operands are fp16 (full PE rate at any moving-dim size; the
f32r path costs 4 cycles/row below 256 moving), accumulation stays
fp32 in PSUM. x is pre-masked by the padding keep-mask on the host, so
q/k/v are zeroed for padded tokens with no on-device mask multiplies
(with nonzero qkv-bias the bias lane row is the keep vector, so the
post-projection masked_fill is still exact). exp uses a constant -4
shift (cancels in the softmax normalization) so probabilities stay
comfortably inside fp16 range.

Per-core dataflow, one merged 18-step loop that interleaves projection
chains with attention so PE (~89% busy), ACT, DVE, and GPSIMD stay
balanced; per step t: scores for key block t-1 (4 heads, spaced
through the step), one qk^T chain quarter (steps 2-13), v projection
of block t-1, AV + finish of query block t-2:
  - qk^T chains: qkp[128ch, 512tok] += wq_i^T x_i over 8 row-tiles,
    evicted to fp16 SBUF (DVE copy). The first quarter runs
    row-pair-major across all 4 chains in a dedicated 4-bank PSUM
    prefix pool, chasing the 8 interleaved wq/x row-pair DMAs.
  - v chains per token block: v[128tok, 256ch], evicted (ACT; DVE for
    the last blocks) with an appended ones lane per head.
  - scores per key block kb: [128k, <=384q] = K_slice^T Q_window,
    exp((s)/8 - 4) on ACT, band01 multiply on DVE (head 3 on GPSIMD).
  - AV per query block: [128q, 4*65] accumulating 3 key blocks; col 64
    is the softmax denominator. normalize = DVE reciprocal + one
    broadcast tensor_tensor multiply (stride-0 free dim).
  - PE transposes vals, o-proj partials accumulate per 512-half into
    two single-bank PSUM tiles; half 0 is ACT-evicted immediately,
    half 1's eviction is deferred into the next block's finish (ACT
    for the last blocks) to avoid DVE head-of-line blocking; fp16
    partials are DMA'd out per block pair.
  - the last two blocks stage/evict/DMA per half with their own tiles
    (block 15's first o-proj group gets the PSUM bank freed by the
    chain pool) so the kernel tail is one half-eviction + small DMA.

Inputs are loaded as a few large [128, ntiles, cols] DMAs; PE
"toucher" matmuls absorb the prefix DMA semaphores one at a time so
chain matmuls carry at most one inline wait.

Cost-model timeline (CoreSim): 79917 ns vs 118130 ns for the f32r
baseline; hardware rel err ~5.8e-4.
"""

import os

import numpy as np

B = 2
S = 2048
IN_DIM = 1024
EMBED = 1024
HEADS = 16
WINDOW = 256
HD = 64
H_LOC = 4          # heads per core
N_CORES = 8
QK_CH = 2 * H_LOC * HD   # 512
V_CH = H_LOC * HD        # 256
NB = S // 128            # 16 token blocks
VW = HD + 1              # value channels + softmax denominator lane
EXP_SHIFT = -4.0         # exp(s/8 - 4): cancels in softmax, keeps fp16 finite

_CACHE = {}
LAST = {"exec_time_ns": None, "results": None}


def _build_nc(zero_bias):
    import concourse.bass as bass
    import concourse.mybir as mybir
    import concourse.tile as tile
    from concourse import bacc
    from concourse.masks import make_identity
    from contextlib import ExitStack

    F32 = mybir.dt.float32
    FP16 = mybir.dt.float16
    AP = bass.AP
    KT = 8 if zero_bias else 9
    IN_ROWS = IN_DIM if zero_bias else IN_DIM + 1
    ACT_EXP = mybir.ActivationFunctionType.Exp

    nc = bacc.Bacc()
    xT = nc.dram_tensor("xT", [IN_ROWS, S], FP16, kind="ExternalInput")
    wqkT = nc.dram_tensor("wqkT", [IN_ROWS, QK_CH], FP16, kind="ExternalInput")
    wvT = nc.dram_tensor("wvT", [IN_ROWS, V_CH], FP16, kind="ExternalInput")
    woT = nc.dram_tensor("woT", [V_CH, EMBED], FP16, kind="ExternalInput")
    mask01 = nc.dram_tensor("mask01", [128, 384], FP16, kind="ExternalInput")
    out = nc.dram_tensor("out", [S, EMBED], FP16, kind="ExternalOutput")

    with tile.TileContext(nc) as tc, ExitStack() as es:
        main = es.enter_context(tc.tile_pool(name="main", bufs=1))
        ident = main.tile([128, 128], FP16)
        make_identity(nc, ident)
        mk = main.tile([128, 384], FP16)
        xt = main.tile([128, 8, S], FP16, name="xt")
        wq = main.tile([128, 8, QK_CH], FP16, name="wq")
        wv = main.tile([128, 8, V_CH], FP16, name="wv")
        wo = main.tile([128, 2, EMBED], FP16, name="wo")
        qk = [main.tile([128, S], FP16, name=f"qk{c}") for c in range(4)]
        v_all = main.tile([128, NB, H_LOC, VW], FP16, name="v")
        nc.vector.memset(v_all[:, :, :, HD:VW], 1.0)
        ebias = main.tile([128, 1], F32)
        nc.vector.memset(ebias, EXP_SHIFT)
        if KT == 9:
            xt9 = main.tile([1, S], FP16, name="xt9")
            wq9 = main.tile([1, QK_CH], FP16, name="wq9")
            wv9 = main.tile([1, V_CH], FP16, name="wv9")

        def XT(i, c0, c1):
            return xt[:, i, c0:c1] if i < 8 else xt9[:, c0:c1]

        def WQ(i, c0, c1):
            return wq[:, i, c0:c1] if i < 8 else wq9[:, c0:c1]

        def WV(i):
            return wv[:, i, :] if i < 8 else wv9[:, :]

        # --- input DMAs (SP queue). First wq/x quarter interleaved in row
        # pairs so projection chains start early; everything else as single
        # multi-tile transfers. ---
        xTt = xT.ap().tensor

        def pair_dma(i):
            nc.sync.dma_start(
                out=wq[:, 2 * i : 2 * i + 2, :],
                in_=AP(
                    tensor=wqkT.ap().tensor,
                    offset=256 * i * QK_CH,
                    ap=[[QK_CH, 128], [128 * QK_CH, 2], [1, QK_CH]],
                ),
            )
            nc.sync.dma_start(
                out=xt[:, 2 * i : 2 * i + 2, 0:512],
                in_=AP(
                    tensor=xTt,
                    offset=256 * i * S,
                    ap=[[S, 128], [128 * S, 2], [1, 512]],
                ),
            )

        def xq_dma(tq):
            nc.sync.dma_start(
                out=xt[:, :, 512 * tq : 512 * (tq + 1)],
                in_=AP(
                    tensor=xTt,
                    offset=512 * tq,
                    ap=[[S, 128], [128 * S, 8], [1, 512]],
                ),
            )

        for i in range(4):
            pair_dma(i)
        xq_dma(1)
        nc.sync.dma_start(out=mk, in_=mask01[:, :])
        nc.sync.dma_start(
            out=wv[:, :, :],
            in_=AP(
                tensor=wvT.ap().tensor,
                offset=0,
                ap=[[V_CH, 128], [128 * V_CH, 8], [1, V_CH]],
            ),
        )
        if KT == 9:
            nc.sync.dma_start(out=xt9, in_=xT[IN_DIM : IN_DIM + 1, :])
            nc.sync.dma_start(out=wq9, in_=wqkT[IN_DIM : IN_DIM + 1, :])
            nc.sync.dma_start(out=wv9, in_=wvT[IN_DIM : IN_DIM + 1, :])
        nc.sync.dma_start(
            out=wo[:, :, :],
            in_=AP(
                tensor=woT.ap().tensor,
                offset=0,
                ap=[[EMBED, 128], [128 * EMBED, 2], [1, EMBED]],
            ),
        )
        xq_dma(2)
        xq_dma(3)

        def qk_chain_part(c, tq, qkp, i0, i1):
            for i in range(i0, i1):
                nc.tensor.matmul(
                    qkp[:, :],
                    WQ(i, 128 * c, 128 * (c + 1)),
                    XT(i, 512 * tq, 512 * (tq + 1)),
                    start=(i == 0),
                    stop=(i == KT - 1),
                )

        def qk_evict(c, tq, qkp):
            nc.vector.tensor_copy(qk[c][:, 512 * tq : 512 * (tq + 1)], qkp[:, :])

        # --- prefix: PE touchers absorb the 8 interleaved prefix DMA
        # semaphores one at a time, then the 4 quarter-0 qk chains run
        # tile-pair-major (all 4 chains advance as each row-pair DMA lands)
        # in a dedicated 4-bank PSUM pool that closes before the main loop's
        # pools open. ---
        with tc.tile_pool(name="tch_ps", bufs=1, space="PSUM") as tchps, tc.tile_pool(
            name="pqk_ps", bufs=1, space="PSUM"
        ) as pqkps:
            tch = tchps.tile([1, 8], F32)
            for i in range(4):
                for t_ap in (wq[0:1, 2 * i, 0:1], xt[0:1, 2 * i, 0:1]):
                    nc.tensor.matmul(tch[:, 0:1], t_ap, t_ap, start=True, stop=True)
            pq = [
                pqkps.tile([128, 512], F32, name=f"pqk{c}", tag=f"pq{c}")
                for c in range(4)
            ]
            # c-order puts the chains feeding the first score heads (k of
            # heads 0/1 = c2, q of heads 0/1 = c0) first; each chain is
            # evicted right after its final row-pair so DVE overlaps PE.
            CORD = (2, 0, 3, 1)
            for pair in range(3):
                for c in CORD:
                    qk_chain_part(c, 0, pq[c], 2 * pair, 2 * pair + 2)
            for n, c in enumerate(CORD):
                qk_chain_part(c, 0, pq[c], 6, KT)
                if n % 2 == 0:
                    qk_evict(c, 0, pq[c])
                else:
                    nc.scalar.copy(qk[c][:, 0:512], pq[c][:, :])

        with tc.tile_pool(
            name="v_ps", bufs=1, space="PSUM"
        ) as vps, tc.tile_pool(name="sc_ps", bufs=2, space="PSUM") as scps, tc.tile_pool(
            name="av_ps", bufs=1, space="PSUM"
        ) as avps, tc.tile_pool(name="tp_ps", bufs=1, space="PSUM") as tpps, tc.tile_pool(
            name="wk", bufs=16
        ) as wk, tc.tile_pool(name="wk2", bufs=3) as wk2, tc.tile_pool(
            name="ot", bufs=2
        ) as otp:
            P = {}

            def v_proj(b2):
                vp = vps.tile([128, V_CH], F32, name=f"vp{b2}", tag="vp")
                for i in range(KT):
                    nc.tensor.matmul(
                        vp[:, :],
                        XT(i, 128 * b2, 128 * (b2 + 1)),
                        WV(i),
                        start=(i == 0),
                        stop=(i == KT - 1),
                    )
                (nc.vector.tensor_copy if b2 >= 11 else nc.scalar.copy)(
                    v_all[:, b2, :, 0:HD],
                    vp[:, :].rearrange("p (h d) -> p h d", d=HD),
                )

            def sc_head(kb, h):
                qlo = max(0, 128 * (kb - 1))
                qhi = min(S, 128 * (kb + 2))
                qw = qhi - qlo
                moff = qlo - 128 * (kb - 1)
                ct = 2 + h // 2
                pbase = 64 * (h % 2)
                sc = scps.tile([128, 512], F32, name=f"sc{kb}_{h}", tag="sc")
                nc.tensor.matmul(
                    sc[:, 0:qw],
                    qk[ct][pbase : pbase + 64, 128 * kb : 128 * (kb + 1)],
                    qk[h // 2][pbase : pbase + 64, qlo:qhi],
                    start=True,
                    stop=True,
                )
                p = wk.tile([128, 384], FP16, name=f"p{kb}_{h}", tag="p")
                nc.scalar.activation(
                    p[:, 0:qw], sc[:, 0:qw], func=ACT_EXP, bias=ebias[:, :],
                    scale=0.125,
                )
                # band mask: odd heads on GPSIMD (otherwise idle) to keep DVE
                # off the critical path
                eng = nc.gpsimd if h == 3 else nc.vector
                eng.tensor_mul(p[:, 0:qw], p[:, 0:qw], mk[:, moff : moff + qw])
                P[(kb, h)] = p

            def av_head(qb, a, h):
                kbs = [k2 for k2 in (qb - 1, qb, qb + 1) if 0 <= k2 < NB]
                for idx, k2 in enumerate(kbs):
                    off = 128 * qb - max(0, 128 * (k2 - 1))
                    nc.tensor.matmul(
                        a[:, VW * h : VW * h + VW],
                        P[(k2, h)][:, off : off + 128],
                        v_all[:, k2, h, :],
                        start=(idx == 0),
                        stop=(idx == len(kbs) - 1),
                    )

            def emit_defB():
                # previous block's half-1 eviction (+pair DMA), deferred past
                # this block's normalize chain to avoid DVE head-of-line
                # blocking on the o-proj PSUM dependency
                if defB[0] is not None:
                    d_op, d_ot, d_qb = defB[0]
                    # late deferred evictions go to ACT (idle once exps are
                    # done) so they never head-of-line block DVE's normalize
                    ev = nc.scalar.copy if d_qb >= 13 else nc.vector.tensor_copy
                    ev(d_ot[:, d_qb % 2, 512:1024], d_op[:, :])
                    if d_qb % 2 == 1:
                        nc.sync.dma_start(
                            out=AP(
                                tensor=out.ap().tensor,
                                offset=128 * (d_qb - 1) * EMBED,
                                ap=[[EMBED, 128], [128 * EMBED, 2], [1, EMBED]],
                            ),
                            in_=d_ot[:, :, :],
                        )
                    defB[0] = None

            def finish_block(qb, a, opps, ot_box):
                a3 = a.rearrange("p (h c) -> p h c", c=VW)
                recip = wk2.tile([128, H_LOC, 1], F32, name=f"rc{qb}", tag="rc")
                nc.vector.reciprocal(recip, a3[:, :, HD:VW])
                vals = wk2.tile([128, H_LOC, HD], FP16, name=f"vl{qb}", tag="vl")
                rap = recip[:, :, :]
                rbc = bass.AP(
                    tensor=rap.tensor,
                    offset=rap.offset,
                    ap=[rap.ap[0], rap.ap[1], [0, HD]],
                )
                nc.vector.tensor_mul(vals, a3[:, :, 0:HD], rbc)
                vals2 = vals.rearrange("p h c -> p (h c)")
                tp = tpps.tile([128, 2, 128], FP16, name=f"tp{qb}", tag="tp")
                vTs = []
                for c2 in range(2):
                    nc.tensor.transpose(
                        tp[:, c2, :], vals2[:, 128 * c2 : 128 * (c2 + 1)], ident[:, :]
                    )
                    vT = wk2.tile([128, 128], FP16, name=f"vT{qb}_{c2}", tag=f"vT{c2}")
                    nc.vector.tensor_copy(vT[:, :], tp[:, c2, :])
                    vTs.append(vT)
                emit_defB()
                ops = [
                    opps.tile([128, 512], F32, name=f"op{qb}_{n2}", tag=f"op{n2}")
                    for n2 in range(2)
                ]
                if qb >= NB - 2:
                    # block 14: own staging tile, half-0 evicted+DMA'd right
                    # away; half-1's eviction is deferred into finish15 so it
                    # doesn't sit in the DVE queue ahead of block 15's
                    # normalize chain.
                    otl = otp.tile([128, EMBED], FP16, name=f"otl{qb}", tag="otl")
                    for n2 in range(2):
                        for c2 in range(2):
                            nc.tensor.matmul(
                                ops[n2][:, :],
                                vTs[c2][:, :],
                                wo[:, c2, 512 * n2 : 512 * (n2 + 1)],
                                start=(c2 == 0),
                                stop=(c2 == 1),
                            )
                        if n2 == 0:
                            nc.scalar.copy(otl[:, 0:512], ops[0][:, :])
                            nc.sync.dma_start(
                                out=AP(
                                    tensor=out.ap().tensor,
                                    offset=128 * qb * EMBED,
                                    ap=[[EMBED, 128], [1, 512]],
                                ),
                                in_=otl[:, 0:512],
                            )
                    def14[0] = (ops[1], otl)
                    return
                if qb % 2 == 0:
                    ot_box[0] = otp.tile(
                        [128, 2, EMBED], FP16, name=f"ot{qb}", tag="ot"
                    )
                for n2 in range(2):
                    for c2 in range(2):
                        nc.tensor.matmul(
                            ops[n2][:, :],
                            vTs[c2][:, :],
                            wo[:, c2, 512 * n2 : 512 * (n2 + 1)],
                            start=(c2 == 0),
                            stop=(c2 == 1),
                        )
                    if n2 == 0:
                        # evict group 0 on ACT as soon as it stops (overlaps
                        # group 1's matmuls); group 1's DVE eviction and the
                        # pair DMA are deferred into the next block's finish
                        nc.scalar.copy(ot_box[0][:, qb % 2, 0:512], ops[0][:, :])
                defB[0] = (ops[1], ot_box[0], qb)

            def finish15(qb, opps, op15):
                # last block: fully pipelined per head pair so the tail is
                # AV(h2,h3) -> norm -> transpose -> o-proj stop -> half
                # eviction -> half DMA with no long serial chain.
                a = avps.tile([128, H_LOC * VW], F32, name=f"av{qb}", tag="av")
                a3 = a.rearrange("p (h c) -> p h c", c=VW)
                tp = tpps.tile([128, 2, 128], FP16, name=f"tp{qb}", tag="tp")
                ops = [
                    op15.tile([128, 512], F32, name=f"op{qb}_0", tag="op15"),
                    opps.tile([128, 512], F32, name=f"op{qb}_1", tag="op0"),
                ]
                otl = otp.tile([128, EMBED], FP16, name=f"otl{qb}", tag="otl")
                for h in range(4):
                    av_head(qb, a, h)
                recip = wk2.tile([128, H_LOC, 1], F32, name=f"rc{qb}", tag="rc")
                nc.vector.reciprocal(recip, a3[:, :, HD:VW])
                vals = wk2.tile([128, H_LOC, HD], FP16, name=f"vl{qb}", tag="vl")
                rap = recip[:, :, :]
                rbc = bass.AP(
                    tensor=rap.tensor,
                    offset=rap.offset,
                    ap=[rap.ap[0], rap.ap[1], [0, HD]],
                )
                nc.vector.tensor_mul(vals, a3[:, :, 0:HD], rbc)
                vals2 = vals.rearrange("p h c -> p (h c)")
                vTs = []
                for c2 in range(2):
                    nc.tensor.transpose(
                        tp[:, c2, :], vals2[:, 128 * c2 : 128 * (c2 + 1)],
                        ident[:, :],
                    )
                    vT = wk2.tile(
                        [128, 128], FP16, name=f"vT{qb}_{c2}", tag=f"vT{c2}"
                    )
                    nc.vector.tensor_copy(vT[:, :], tp[:, c2, :])
                    vTs.append(vT)
                if def14[0] is not None:
                    d_op, d_otl = def14[0]
                    nc.scalar.copy(d_otl[:, 512:1024], d_op[:, :])
                    nc.sync.dma_start(
                        out=AP(
                            tensor=out.ap().tensor,
                            offset=128 * (NB - 2) * EMBED + 512,
                            ap=[[EMBED, 128], [1, 512]],
                        ),
                        in_=d_otl[:, 512:1024],
                    )
                for c2 in range(2):
                    for n2 in range(2):
                        nc.tensor.matmul(
                            ops[n2][:, :],
                            vTs[c2][:, :],
                            wo[:, c2, 512 * n2 : 512 * (n2 + 1)],
                            start=(c2 == 0),
                            stop=(c2 == 1),
                        )
                for n2 in range(2):
                    ev = nc.scalar.copy if n2 == 0 else nc.vector.tensor_copy
                    ev(otl[:, 512 * n2 : 512 * (n2 + 1)], ops[n2][:, :])
                    dma_eng = nc.sync if n2 == 0 else nc.scalar
                    dma_eng.dma_start(
                        out=AP(
                            tensor=out.ap().tensor,
                            offset=128 * qb * EMBED + 512 * n2,
                            ap=[[EMBED, 128], [1, 512]],
                        ),
                        in_=otl[:, 512 * n2 : 512 * (n2 + 1)],
                    )

            with tc.tile_pool(name="op_ps", bufs=1, space="PSUM") as opps:
                CH = [(c, tq) for tq in range(1, 4) for c in range(4)]
                ot_box = [None]
                a_cur = [None]
                def14 = [None]
                defB = [None]

                def step(t, qkps, op15=None):
                    kb = t - 1
                    qb = t - 2
                    chain = CH[t - 2] if 2 <= t <= 13 else None
                    qkp = None
                    if chain is not None:
                        qkp = qkps.tile(
                            [128, 512], F32, name=f"qkp{chain[0]}_{chain[1]}",
                            tag="qkp",
                        )
                    if 0 <= kb < NB:
                        sc_head(kb, 0)
                    if chain is not None:
                        qk_chain_part(*chain, qkp, 0, KT // 2)
                    if 0 <= kb < NB:
                        sc_head(kb, 1)
                    if chain is not None:
                        qk_chain_part(*chain, qkp, KT // 2, KT)
                        qk_evict(*chain, qkp)
                    if 1 <= t <= NB:
                        v_proj(t - 1)
                    if 0 <= kb < NB:
                        sc_head(kb, 2)
                    if 0 <= qb < NB - 1:
                        a_cur[0] = avps.tile(
                            [128, H_LOC * VW], F32, name=f"av{qb}", tag="av"
                        )
                        for h in range(3):
                            av_head(qb, a_cur[0], h)
                    if 0 <= kb < NB:
                        sc_head(kb, 3)
                    if 0 <= qb < NB - 1:
                        av_head(qb, a_cur[0], 3)
                        finish_block(qb, a_cur[0], opps, ot_box)
                    elif qb == NB - 1:
                        finish15(qb, opps, op15)

                # the projection-chain PSUM bank is only live through step
                # 13; closing its pool there frees a bank that block 15's
                # first o-proj group can use without waiting on block 14's
                # eviction.
                with tc.tile_pool(name="qk_ps", bufs=1, space="PSUM") as qkps:
                    for t in range(14):
                        step(t, qkps)
                with tc.tile_pool(name="op15_ps", bufs=1, space="PSUM") as op15:
                    for t in range(14, NB + 2):
                        step(t, None, op15)

    return nc


def _get_nc(zero_bias):
    key = ("nc", bool(zero_bias))
    if key not in _CACHE:
        nc = _build_nc(zero_bias)
        nc.finalize()
        _CACHE[key] = nc
    return _CACHE[key]


def _prep_in_maps(x, padding_mask, Wqkv, bqkv, Wo, bo):
    f16 = np.float16
    x = np.asarray(x, dtype=np.float32)
    pm = np.asarray(padding_mask)
    Wqkv = np.asarray(Wqkv, dtype=np.float32)
    bqkv = np.asarray(bqkv, dtype=np.float32)
    Wo = np.asarray(Wo, dtype=np.float32)
    zero_bias = not bqkv.any()
    rows = IN_DIM if zero_bias else IN_DIM + 1

    keep_b = (pm == 0).astype(np.float32)  # [B, S]
    xT_b = []
    for b in range(B):
        aug = np.zeros((rows, S), dtype=f16)
        aug[:IN_DIM] = (x[b] * keep_b[b][:, None]).T.astype(f16)
        if not zero_bias:
            aug[IN_DIM] = keep_b[b].astype(f16)
        xT_b.append(aug)

    # band mask tile: mask[p, n] = 1 iff 0 <= n - p <= 256
    p_ = np.arange(128)[:, None]
    n_ = np.arange(384)[None, :]
    d = n_ - p_
    mask01 = ((d >= 0) & (d <= WINDOW)).astype(f16)

    in_maps = []
    for c in range(N_CORES):
        b = c // 4
        g = c % 4
        heads = [4 * g + j for j in range(H_LOC)]
        q_rows = np.concatenate([Wqkv[192 * h : 192 * h + 64] for h in heads])
        k_rows = np.concatenate([Wqkv[192 * h + 64 : 192 * h + 128] for h in heads])
        v_rows = np.concatenate([Wqkv[192 * h + 128 : 192 * h + 192] for h in heads])

        wqkT = np.zeros((rows, QK_CH), dtype=f16)
        wqkT[:IN_DIM] = np.concatenate([q_rows, k_rows]).T.astype(f16)
        wvT = np.zeros((rows, V_CH), dtype=f16)
        wvT[:IN_DIM] = v_rows.T.astype(f16)
        if not zero_bias:
            bq = np.concatenate([bqkv[192 * h : 192 * h + 64] for h in heads])
            bk = np.concatenate([bqkv[192 * h + 64 : 192 * h + 128] for h in heads])
            bv = np.concatenate([bqkv[192 * h + 128 : 192 * h + 192] for h in heads])
            wqkT[IN_DIM] = np.concatenate([bq, bk]).astype(f16)
            wvT[IN_DIM] = bv.astype(f16)
        woT = np.ascontiguousarray(Wo[:, 256 * g : 256 * (g + 1)].T.astype(f16))

        in_maps.append(
            {
                "xT": xT_b[b],
                "wqkT": wqkT,
                "wvT": wvT,
                "woT": woT,
                "mask01": mask01,
            }
        )
    return in_maps


def kernel(x, padding_mask, Wqkv, bqkv, Wo, bo):
    from concourse.bass_utils import run_bass_kernel_spmd

    zero_bias = not np.asarray(bqkv, dtype=np.float32).any()
    nc = _get_nc(zero_bias)
    in_maps = _prep_in_maps(x, padding_mask, Wqkv, bqkv, Wo, bo)
    trace = bool(int(os.environ.get("KERNEL_TRACE", "0")))
    res = run_bass_kernel_spmd(nc, in_maps, list(range(N_CORES)), trace=trace)
    LAST["exec_time_ns"] = res.exec_time_ns
    LAST["results"] = res

    bo = np.asarray(bo, dtype=np.float32)
    out = np.zeros((B, S, EMBED), dtype=np.float32)
    for c in range(N_CORES):
        out[c // 4] += res.results[c]["out"].astype(np.float32)
    out += bo[None, None, :]
    return out


# revision 67
# speedup vs baseline: 1.0018x; 1.0018x over previous
"""Banded multi-head attention (window=256) on 8 Trainium2 NeuronCores.

Sharding: core c handles batch b = c // 4 and head group g = c % 4
(4 of 16 heads). QKV projection is column-sharded per head group, the
banded attention is embarrassingly parallel over (batch, head), and the
output projection is row-sharded (each core produces a partial [S, E]
output in fp16; the host sums the 4 partials per batch and adds the
bias).

All matmul operands are fp16 (full PE rate at any moving-dim size; the
f32r path costs 4 cycles/row below 256 moving), accumulation stays
fp32 in PSUM. x is pre-masked by the padding keep-mask on the host, so
q/k/v are zeroed for padded tokens with no on-device mask multiplies
(with nonzero qkv-bias the bias lane row is the keep vector, so the
post-projection masked_fill is still exact). exp uses a constant -4
shift (cancels in the softmax normalization) so probabilities stay
comfortably inside fp16 range.

Per-core dataflow, one merged 18-step loop that interleaves projection
chains with attention so PE (~89% busy), ACT, DVE, and GPSIMD stay
balanced; per step t: scores for key block t-1 (4 heads, spaced
through the step), one qk^T chain quarter (steps 2-13), v projection
of block t-1, AV + finish of query block t-2:
  - qk^T chains: qkp[128ch, 512tok] += wq_i^T x_i over 8 row-tiles,
    evicted to fp16 SBUF (DVE copy). The first quarter runs
    row-pair-major across all 4 chains in a dedicated 4-bank PSUM
    prefix pool, chasing the 8 interleaved wq/x row-pair DMAs.
  - v chains per token block: v[128tok, 256ch], evicted (ACT; DVE for
    the last blocks) with an appended ones lane per head.
  - scores per key block kb: [128k, <=384q] = K_slice^T Q_window,
    exp((s)/8 - 4) on ACT, band01 multiply on DVE (head 3 on GPSIMD).
  - AV per query block: [128q, 4*65] accumulating 3 key blocks; col 64
    is the softmax denominator. normalize = DVE reciprocal + one
    broadcast tensor_tensor multiply (stride-0 free dim).
  - PE transposes vals, o-proj partials accumulate per 512-half into
    two single-bank PSUM tiles; half 0 is ACT-evicted immediately,
    half 1's eviction is deferred into the next block's finish (ACT
    for the last blocks) to avoid DVE head-of-line blocking; fp16
    partials are DMA'd out per block pair.
  - the last two blocks stage/evict/DMA per half with their own tiles
    (block 15's first o-proj group gets the PSUM bank freed by the
    chain pool) so the kernel tail is one half-eviction + small DMA.

Inputs are loaded as a few large [128, ntiles, cols] DMAs; PE
"toucher" matmuls absorb the prefix DMA semaphores one at a time so
chain matmuls carry at most one inline wait.

Cost-model timeline (CoreSim): 79771 ns vs 118130 ns for the f32r
baseline; hardware rel err ~5.8e-4.
"""

import os

import numpy as np

B = 2
S = 2048
IN_DIM = 1024
EMBED = 1024
HEADS = 16
WINDOW = 256
HD = 64
H_LOC = 4          # heads per core
N_CORES = 8
QK_CH = 2 * H_LOC * HD   # 512
V_CH = H_LOC * HD        # 256
NB = S // 128            # 16 token blocks
VW = HD + 1              # value channels + softmax denominator lane
EXP_SHIFT = -4.0         # exp(s/8 - 4): cancels in softmax, keeps fp16 finite

_CACHE = {}
LAST = {"exec_time_ns": None, "results": None}


def _build_nc(zero_bias):
    import concourse.bass as bass
    import concourse.mybir as mybir
    import concourse.tile as tile
    from concourse import bacc
    from concourse.masks import make_identity
    from contextlib import ExitStack

    F32 = mybir.dt.float32
    FP16 = mybir.dt.float16
    AP = bass.AP
    KT = 8 if zero_bias else 9
    IN_ROWS = IN_DIM if zero_bias else IN_DIM + 1
    ACT_EXP = mybir.ActivationFunctionType.Exp

    nc = bacc.Bacc()
    xT = nc.dram_tensor("xT", [IN_ROWS, S], FP16, kind="ExternalInput")
    wqkT = nc.dram_tensor("wqkT", [IN_ROWS, QK_CH], FP16, kind="ExternalInput")
    wvT = nc.dram_tensor("wvT", [IN_ROWS, V_CH], FP16, kind="ExternalInput")
    woT = nc.dram_tensor("woT", [V_CH, EMBED], FP16, kind="ExternalInput")
    mask01 = nc.dram_tensor("mask01", [128, 384], FP16, kind="ExternalInput")
    out = nc.dram_tensor("out", [S, EMBED], FP16, kind="ExternalOutput")

    with tile.TileContext(nc) as tc, ExitStack() as es:
        main = es.enter_context(tc.tile_pool(name="main", bufs=1))
        ident = main.tile([128, 128], FP16)
        make_identity(nc, ident)
        mk = main.tile([128, 384], FP16)
        xt = main.tile([128, 8, S], FP16, name="xt")
        wq = main.tile([128, 8, QK_CH], FP16, name="wq")
        wv = main.tile([128, 8, V_CH], FP16, name="wv")
        wo = main.tile([128, 2, EMBED], FP16, name="wo")
        qk = [main.tile([128, S], FP16, name=f"qk{c}") for c in range(4)]
        v_all = main.tile([128, NB, H_LOC, VW], FP16, name="v")
        nc.vector.memset(v_all[:, :, :, HD:VW], 1.0)
        ebias = main.tile([128, 1], F32)
        nc.vector.memset(ebias, EXP_SHIFT)
        if KT == 9:
            xt9 = main.tile([1, S], FP16, name="xt9")
            wq9 = main.tile([1, QK_CH], FP16, name="wq9")
            wv9 = main.tile([1, V_CH], FP16, name="wv9")

        def XT(i, c0, c1):
            return xt[:, i, c0:c1] if i < 8 else xt9[:, c0:c1]

        def WQ(i, c0, c1):
            return wq[:, i, c0:c1] if i < 8 else wq9[:, c0:c1]

        def WV(i):
            return wv[:, i, :] if i < 8 else wv9[:, :]

        # --- input DMAs (SP queue). First wq/x quarter interleaved in row
        # pairs so projection chains start early; everything else as single
        # multi-tile transfers. ---
        xTt = xT.ap().tensor

        def pair_dma(i):
            nc.sync.dma_start(
                out=wq[:, 2 * i : 2 * i + 2, :],
                in_=AP(
                    tensor=wqkT.ap().tensor,
                    offset=256 * i * QK_CH,
                    ap=[[QK_CH, 128], [128 * QK_CH, 2], [1, QK_CH]],
                ),
            )
            nc.sync.dma_start(
                out=xt[:, 2 * i : 2 * i + 2, 0:512],
                in_=AP(
                    tensor=xTt,
                    offset=256 * i * S,
                    ap=[[S, 128], [128 * S, 2], [1, 512]],
                ),
            )

        def xq_dma(tq):
            nc.sync.dma_start(
                out=xt[:, :, 512 * tq : 512 * (tq + 1)],
                in_=AP(
                    tensor=xTt,
                    offset=512 * tq,
                    ap=[[S, 128], [128 * S, 8], [1, 512]],
                ),
            )

        for i in range(4):
            pair_dma(i)
        xq_dma(1)
        nc.sync.dma_start(out=mk, in_=mask01[:, :])
        nc.sync.dma_start(
            out=wv[:, :, :],
            in_=AP(
                tensor=wvT.ap().tensor,
                offset=0,
                ap=[[V_CH, 128], [128 * V_CH, 8], [1, V_CH]],
            ),
        )
        if KT == 9:
            nc.sync.dma_start(out=xt9, in_=xT[IN_DIM : IN_DIM + 1, :])
            nc.sync.dma_start(out=wq9, in_=wqkT[IN_DIM : IN_DIM + 1, :])
            nc.sync.dma_start(out=wv9, in_=wvT[IN_DIM : IN_DIM + 1, :])
        nc.sync.dma_start(
            out=wo[:, :, :],
            in_=AP(
                tensor=woT.ap().tensor,
                offset=0,
                ap=[[EMBED, 128], [128 * EMBED, 2], [1, EMBED]],
            ),
        )
        xq_dma(2)
        xq_dma(3)

        def qk_chain_part(c, tq, qkp, i0, i1):
            for i in range(i0, i1):
                nc.tensor.matmul(
                    qkp[:, :],
                    WQ(i, 128 * c, 128 * (c + 1)),
                    XT(i, 512 * tq, 512 * (tq + 1)),
                    start=(i == 0),
                    stop=(i == KT - 1),
                )

        def qk_evict(c, tq, qkp):
            nc.vector.tensor_copy(qk[c][:, 512 * tq : 512 * (tq + 1)], qkp[:, :])

        # --- prefix: PE touchers absorb the 8 interleaved prefix DMA
        # semaphores one at a time, then the 4 quarter-0 qk chains run
        # tile-pair-major (all 4 chains advance as each row-pair DMA lands)
        # in a dedicated 4-bank PSUM pool that closes before the main loop's
        # pools open. ---
        with tc.tile_pool(name="tch_ps", bufs=1, space="PSUM") as tchps, tc.tile_pool(
            name="pqk_ps", bufs=1, space="PSUM"
        ) as pqkps:
            tch = tchps.tile([1, 8], F32)
            for i in range(4):
                for t_ap in (wq[0:1, 2 * i, 0:1], xt[0:1, 2 * i, 0:1]):
                    nc.tensor.matmul(tch[:, 0:1], t_ap, t_ap, start=True, stop=True)
            pq = [
                pqkps.tile([128, 512], F32, name=f"pqk{c}", tag=f"pq{c}")
                for c in range(4)
            ]
            # c-order puts the chains feeding the first score heads (k of
            # heads 0/1 = c2, q of heads 0/1 = c0) first; each chain is
            # evicted right after its final row-pair so DVE overlaps PE.
            CORD = (2, 0, 3, 1)
            for pair in range(3):
                for c in CORD:
                    qk_chain_part(c, 0, pq[c], 2 * pair, 2 * pair + 2)
            for n, c in enumerate(CORD):
                qk_chain_part(c, 0, pq[c], 6, KT)
                if n % 2 == 0:
                    qk_evict(c, 0, pq[c])
                else:
                    nc.scalar.copy(qk[c][:, 0:512], pq[c][:, :])

        with tc.tile_pool(
            name="v_ps", bufs=1, space="PSUM"
        ) as vps, tc.tile_pool(name="sc_ps", bufs=2, space="PSUM") as scps, tc.tile_pool(
            name="av_ps", bufs=1, space="PSUM"
        ) as avps, tc.tile_pool(name="tp_ps", bufs=1, space="PSUM") as tpps, tc.tile_pool(
            name="wk", bufs=16
        ) as wk, tc.tile_pool(name="wk2", bufs=3) as wk2, tc.tile_pool(
            name="ot", bufs=2
        ) as otp:
            P = {}

            def v_proj(b2):
                vp = vps.tile([128, V_CH], F32, name=f"vp{b2}", tag="vp")
                for i in range(KT):
                    nc.tensor.matmul(
                        vp[:, :],
                        XT(i, 128 * b2, 128 * (b2 + 1)),
                        WV(i),
                        start=(i == 0),
                        stop=(i == KT - 1),
                    )
                (nc.vector.tensor_copy if b2 >= 11 else nc.scalar.copy)(
                    v_all[:, b2, :, 0:HD],
                    vp[:, :].rearrange("p (h d) -> p h d", d=HD),
                )

            def sc_head(kb, h):
                qlo = max(0, 128 * (kb - 1))
                qhi = min(S, 128 * (kb + 2))
                qw = qhi - qlo
                moff = qlo - 128 * (kb - 1)
                ct = 2 + h // 2
                pbase = 64 * (h % 2)
                sc = scps.tile([128, 512], F32, name=f"sc{kb}_{h}", tag="sc")
                nc.tensor.matmul(
                    sc[:, 0:qw],
                    qk[ct][pbase : pbase + 64, 128 * kb : 128 * (kb + 1)],
                    qk[h // 2][pbase : pbase + 64, qlo:qhi],
                    start=True,
                    stop=True,
                )
                p = wk.tile([128, 384], FP16, name=f"p{kb}_{h}", tag="p")
                nc.scalar.activation(
                    p[:, 0:qw], sc[:, 0:qw], func=ACT_EXP, bias=ebias[:, :],
                    scale=0.125,
                )
                # band mask: odd heads on GPSIMD (otherwise idle) to keep DVE
                # off the critical path
                eng = nc.gpsimd if h == 3 else nc.vector
                eng.tensor_mul(p[:, 0:qw], p[:, 0:qw], mk[:, moff : moff + qw])
                P[(kb, h)] = p

            def av_head(qb, a, h):
                kbs = [k2 for k2 in (qb - 1, qb, qb + 1) if 0 <= k2 < NB]
                for idx, k2 in enumerate(kbs):
                    off = 128 * qb - max(0, 128 * (k2 - 1))
                    nc.tensor.matmul(
                        a[:, VW * h : VW * h + VW],
                        P[(k2, h)][:, off : off + 128],
                        v_all[:, k2, h, :],
                        start=(idx == 0),
                        stop=(idx == len(kbs) - 1),
                    )

            def emit_defB():
                # previous block's half-1 eviction (+pair DMA), deferred past
                # this block's normalize chain to avoid DVE head-of-line
                # blocking on the o-proj PSUM dependency
                if defB[0] is not None:
                    d_op, d_ot, d_qb = defB[0]
                    # late deferred evictions go to ACT (idle once exps are
                    # done) so they never head-of-line block DVE's normalize
                    ev = nc.scalar.copy if d_qb >= 13 else nc.vector.tensor_copy
                    ev(d_ot[:, d_qb % 2, 512:1024], d_op[:, :])
                    if d_qb % 2 == 1:
                        nc.sync.dma_start(
                            out=AP(
                                tensor=out.ap().tensor,
                                offset=128 * (d_qb - 1) * EMBED,
                                ap=[[EMBED, 128], [128 * EMBED, 2], [1, EMBED]],
                            ),
                            in_=d_ot[:, :, :],
                        )
                    defB[0] = None

            def finish_block(qb, a, opps, ot_box):
                a3 = a.rearrange("p (h c) -> p h c", c=VW)
                recip = wk2.tile([128, H_LOC, 1], F32, name=f"rc{qb}", tag="rc")
                nc.vector.reciprocal(recip, a3[:, :, HD:VW])
                vals = wk2.tile([128, H_LOC, HD], FP16, name=f"vl{qb}", tag="vl")
                rap = recip[:, :, :]
                rbc = bass.AP(
                    tensor=rap.tensor,
                    offset=rap.offset,
                    ap=[rap.ap[0], rap.ap[1], [0, HD]],
                )
                nc.vector.tensor_mul(vals, a3[:, :, 0:HD], rbc)
                vals2 = vals.rearrange("p h c -> p (h c)")
                tp = tpps.tile([128, 2, 128], FP16, name=f"tp{qb}", tag="tp")
                vTs = []
                for c2 in range(2):
                    nc.tensor.transpose(
                        tp[:, c2, :], vals2[:, 128 * c2 : 128 * (c2 + 1)], ident[:, :]
                    )
                    vT = wk2.tile([128, 128], FP16, name=f"vT{qb}_{c2}", tag=f"vT{c2}")
                    nc.vector.tensor_copy(vT[:, :], tp[:, c2, :])
                    vTs.append(vT)
                emit_defB()
                ops = [
                    opps.tile([128, 512], F32, name=f"op{qb}_{n2}", tag=f"op{n2}")
                    for n2 in range(2)
                ]
                if qb >= NB - 2:
                    # block 14: own staging tile, half-0 evicted+DMA'd right
                    # away; half-1's eviction is deferred into finish15 so it
                    # doesn't sit in the DVE queue ahead of block 15's
                    # normalize chain.
                    otl = otp.tile([128, EMBED], FP16, name=f"otl{qb}", tag="otl")
                    for n2 in range(2):
                        for c2 in range(2):
                            nc.tensor.matmul(
                                ops[n2][:, :],
                                vTs[c2][:, :],
                                wo[:, c2, 512 * n2 : 512 * (n2 + 1)],
                                start=(c2 == 0),
                                stop=(c2 == 1),
                            )
                        if n2 == 0:
                            nc.scalar.copy(otl[:, 0:512], ops[0][:, :])
                            nc.sync.dma_start(
                                out=AP(
                                    tensor=out.ap().tensor,
                                    offset=128 * qb * EMBED,
                                    ap=[[EMBED, 128], [1, 512]],
                                ),
                                in_=otl[:, 0:512],
                            )
                    def14[0] = (ops[1], otl)
                    return
                if qb % 2 == 0:
                    ot_box[0] = otp.tile(
                        [128, 2, EMBED], FP16, name=f"ot{qb}", tag="ot"
                    )
                for n2 in range(2):
                    for c2 in range(2):
                        nc.tensor.matmul(
                            ops[n2][:, :],
                            vTs[c2][:, :],
                            wo[:, c2, 512 * n2 : 512 * (n2 + 1)],
                            start=(c2 == 0),
                            stop=(c2 == 1),
                        )
                    if n2 == 0:
                        # evict group 0 on ACT as soon as it stops (overlaps
                        # group 1's matmuls); group 1's DVE eviction and the
                        # pair DMA are deferred into the next block's finish
                        nc.scalar.copy(ot_box[0][:, qb % 2, 0:512], ops[0][:, :])
                defB[0] = (ops[1], ot_box[0], qb)

            def finish15(qb, opps, op15):
                # last block: fully pipelined per head pair so the tail is
                # AV(h2,h3) -> norm -> transpose -> o-proj stop -> half
                # eviction -> half DMA with no long serial chain.
                a = avps.tile([128, H_LOC * VW], F32, name=f"av{qb}", tag="av")
                a3 = a.rearrange("p (h c) -> p h c", c=VW)
                tp = tpps.tile([128, 2, 128], FP16, name=f"tp{qb}", tag="tp")
                ops = [
                    op15.tile([128, 512], F32, name=f"op{qb}_0", tag="op15"),
                    opps.tile([128, 512], F32, name=f"op{qb}_1", tag="op0"),
                ]
                otl = otp.tile([128, EMBED], FP16, name=f"otl{qb}", tag="otl")
                for h in range(4):
                    av_head(qb, a, h)
                recip = wk2.tile([128, H_LOC, 1], F32, name=f"rc{qb}", tag="rc")
                nc.vector.reciprocal(recip, a3[:, :, HD:VW])
                vals = wk2.tile([128, H_LOC, HD], FP16, name=f"vl{qb}", tag="vl")
                rap = recip[:, :, :]
                rbc = bass.AP(
                    tensor=rap.tensor,
                    offset=rap.offset,
                    ap=[rap.ap[0], rap.ap[1], [0, HD]],
                )
                nc.vector.tensor_mul(vals, a3[:, :, 0:HD], rbc)
                vals2 = vals.rearrange("p h c -> p (h c)")
                vTs = []
                for c2 in range(2):
                    nc.tensor.transpose(
                        tp[:, c2, :], vals2[:, 128 * c2 : 128 * (c2 + 1)],
                        ident[:, :],
                    )
                    vT = wk2.tile(
                        [128, 128], FP16, name=f"vT{qb}_{c2}", tag=f"vT{c2}"
                    )
                    nc.vector.tensor_copy(vT[:, :], tp[:, c2, :])
                    vTs.append(vT)
                if def14[0] is not None:
                    d_op, d_otl = def14[0]
                    nc.scalar.copy(d_otl[:, 512:1024], d_op[:, :])
                    nc.sync.dma_start(
                        out=AP(
                            tensor=out.ap().tensor,
                            offset=128 * (NB - 2) * EMBED + 512,
                            ap=[[EMBED, 128], [1, 512]],
                        ),
                        in_=d_otl[:, 512:1024],
                    )
                for c2 in range(2):
                    for n2 in range(2):
                        nc.tensor.matmul(
                            ops[n2][:, :],
                            vTs[c2][:, :],
                            wo[:, c2, 512 * n2 : 512 * (n2 + 1)],
                            start=(c2 == 0),
                            stop=(c2 == 1),
                        )
                for n2 in range(2):
                    ev = nc.vector.tensor_copy if n2 == 0 else nc.scalar.copy
                    ev(otl[:, 512 * n2 : 512 * (n2 + 1)], ops[n2][:, :])
                    dma_eng = nc.sync if n2 == 0 else nc.scalar
                    dma_eng.dma_start(
                        out=AP(
                            tensor=out.ap().tensor,
                            offset=128 * qb * EMBED + 512 * n2,
                            ap=[[EMBED, 128], [1, 512]],
                        ),
                        in_=otl[:, 512 * n2 : 512 * (n2 + 1)],
                    )

            with tc.tile_pool(name="op_ps", bufs=1, space="PSUM") as opps:
                CH = [(c, tq) for tq in range(1, 4) for c in range(4)]
                ot_box = [None]
                a_cur = [None]
                def14 = [None]
                defB = [None]

                def step(t, qkps, op15=None):
                    kb = t - 1
                    qb = t - 2
                    chain = CH[t - 2] if 2 <= t <= 13 else None
                    qkp = None
                    if chain is not None:
                        qkp = qkps.tile(
                            [128, 512], F32, name=f"qkp{chain[0]}_{chain[1]}",
                            tag="qkp",
                        )
                    if 0 <= kb < NB:
                        sc_head(kb, 0)
                    if chain is not None:
                        qk_chain_part(*chain, qkp, 0, KT // 2)
                    if 0 <= kb < NB:
                        sc_head(kb, 1)
                    if chain is not None:
                        qk_chain_part(*chain, qkp, KT // 2, KT)
                        qk_evict(*chain, qkp)
                    if 1 <= t <= NB:
                        v_proj(t - 1)
                    if 0 <= kb < NB:
                        sc_head(kb, 2)
                    if 0 <= qb < NB - 1:
                        a_cur[0] = avps.tile(
                            [128, H_LOC * VW], F32, name=f"av{qb}", tag="av"
                        )
                        for h in range(3):
                            av_head(qb, a_cur[0], h)
                    if 0 <= kb < NB:
                        sc_head(kb, 3)
                    if 0 <= qb < NB - 1:
                        av_head(qb, a_cur[0], 3)
                        finish_block(qb, a_cur[0], opps, ot_box)
                    elif qb == NB - 1:
                        finish15(qb, opps, op15)

                # the projection-chain PSUM bank is only live through step
                # 13; closing its pool there frees a bank that block 15's
                # first o-proj group can use without waiting on block 14's
                # eviction.
                with tc.tile_pool(name="qk_ps", bufs=1, space="PSUM") as qkps:
                    for t in range(14):
                        step(t, qkps)
                with tc.tile_pool(name="op15_ps", bufs=1, space="PSUM") as op15:
                    for t in range(14, NB + 2):
                        step(t, None, op15)

    return nc


def _get_nc(zero_bias):
    key = ("nc", bool(zero_bias))
    if key not in _CACHE:
        nc = _build_nc(zero_bias)
        nc.finalize()
        _CACHE[key] = nc
    return _CACHE[key]


def _prep_in_maps(x, padding_mask, Wqkv, bqkv, Wo, bo):
    f16 = np.float16
    x = np.asarray(x, dtype=np.float32)
    pm = np.asarray(padding_mask)
    Wqkv = np.asarray(Wqkv, dtype=np.float32)
    bqkv = np.asarray(bqkv, dtype=np.float32)
    Wo = np.asarray(Wo, dtype=np.float32)
    zero_bias = not bqkv.any()
    rows = IN_DIM if zero_bias else IN_DIM + 1

    keep_b = (pm == 0).astype(np.float32)  # [B, S]
    xT_b = []
    for b in range(B):
        aug = np.zeros((rows, S), dtype=f16)
        aug[:IN_DIM] = (x[b] * keep_b[b][:, None]).T.astype(f16)
        if not zero_bias:
            aug[IN_DIM] = keep_b[b].astype(f16)
        xT_b.append(aug)

    # band mask tile: mask[p, n] = 1 iff 0 <= n - p <= 256
    p_ = np.arange(128)[:, None]
    n_ = np.arange(384)[None, :]
    d = n_ - p_
    mask01 = ((d >= 0) & (d <= WINDOW)).astype(f16)

    in_maps = []
    for c in range(N_CORES):
        b = c // 4
        g = c % 4
        heads = [4 * g + j for j in range(H_LOC)]
        q_rows = np.concatenate([Wqkv[192 * h : 192 * h + 64] for h in heads])
        k_rows = np.concatenate([Wqkv[192 * h + 64 : 192 * h + 128] for h in heads])
        v_rows = np.concatenate([Wqkv[192 * h + 128 : 192 * h + 192] for h in heads])

        wqkT = np.zeros((rows, QK_CH), dtype=f16)
        wqkT[:IN_DIM] = np.concatenate([q_rows, k_rows]).T.astype(f16)
        wvT = np.zeros((rows, V_CH), dtype=f16)
        wvT[:IN_DIM] = v_rows.T.astype(f16)
        if not zero_bias:
            bq = np.concatenate([bqkv[192 * h : 192 * h + 64] for h in heads])
            bk = np.concatenate([bqkv[192 * h + 64 : 192 * h + 128] for h in heads])
            bv = np.concatenate([bqkv[192 * h + 128 : 192 * h + 192] for h in heads])
            wqkT[IN_DIM] = np.concatenate([bq, bk]).astype(f16)
            wvT[IN_DIM] = bv.astype(f16)
        woT = np.ascontiguousarray(Wo[:, 256 * g : 256 * (g + 1)].T.astype(f16))

        in_maps.append(
            {
                "xT": xT_b[b],
                "wqkT": wqkT,
                "wvT": wvT,
                "woT": woT,
                "mask01": mask01,
            }
        )
    return in_maps


def kernel(x, padding_mask, Wqkv, bqkv, Wo, bo):
    from concourse.bass_utils import run_bass_kernel_spmd

    zero_bias = not np.asarray(bqkv, dtype=np.float32).any()
    nc = _get_nc(zero_bias)
    in_maps = _prep_in_maps(x, padding_mask, Wqkv, bqkv, Wo, bo)
    trace = bool(int(os.environ.get("KERNEL_TRACE", "0")))
    res = run_bass_kernel_spmd(nc, in_maps, list(range(N_CORES)), trace=trace)
    LAST["exec_time_ns"] = res.exec_time_ns
    LAST["results"] = res

    bo = np.asarray(bo, dtype=np.float32)
    out = np.zeros((B, S, EMBED), dtype=np.float32)
    for c in range(N_CORES):
        out[c // 4] += res.results[c]["out"].astype(np.float32)
    out += bo[None, None, :]
    return out
